# revision 34
# baseline (speedup 1.0000x reference)
"""AttentionClustering (vq_codebook) Trainium2 kernel, 8-core data parallel.

Shard: 8 cores = 4 images x 2 half-images (128 output rows each). Odd cores
get a vertically flipped shard + row-flipped conv weights so every core's
program is identical (true image edge at local top, interior halo at bottom).

Math: q1 = relu(conv3x3(x, w1) + b1); q2 = relu(conv3x3(q1, w2) + b2)  (both
with replicate padding); then the 1x1 conv + cluster-distance softmax folds to
  logit[px, k] = sum_ci q2[ci, px] * muW[k, ci] + bp[k]
  muW = 2 * mu @ W3,  bp = 2 * mu @ b3 - |mu|^2      (|q|^2 cancels in softmax)
  out[px] = sum_k softmax_k(logit) * label[k]

conv2 runs as Winograd F(2,3) over the row (dr) axis: output rows (2p, 2p+1)
come from 4 t-combinations of q1 rows 2p-1..2p+2 (built on DVE), contracted
against G-transformed weights (4t x 3dc x 2kc matmuls of 256 free instead of
direct 2x9 of 512 free -> 2/3 the PE cycles), then the A^T inverse transform
y0 = m0+m1+m2 (DVE) / y1 = m1-m2-m3 (Pool) and the relu+bias on Scalar.
PSUM m-tiles pack t-pairs per 2KB bank, so (t0,t1) and (t2,t3) are emitted as
single 12-matmul accumulation groups (bank zero-regions materialize lazily).
conv1 stays direct fp16; its (2,2) tap is zero-padded to 128 contraction rows
to avoid the 64-row PE tile reconfig penalty. Logits for pair p issue after
conv matmuls of pair p+1 so the PE never waits on the softmax chain.
"""
import sys
if '/opt/trn_rl_repo' not in sys.path:
    sys.path.insert(0, '/opt/trn_rl_repo')

import numpy as np
import concourse.bass as bass
import concourse.mybir as mybir
from concourse import bacc, tile
from concourse.bass_utils import run_bass_kernel_spmd

F32 = mybir.dt.float32
F16 = mybir.dt.float16
AF = mybir.ActivationFunctionType
ALU = mybir.AluOpType
AX = mybir.AxisListType

B, CIN, H, W = 4, 64, 256, 256
Q, K = 256, 16
RB = 32           # output rows per band
NBAND = 4         # bands per core (128 rows)
NCORES = 8

_cached = {}


def build_nc():
    nc = bacc.Bacc("TRN2", target_bir_lowering=False, debug=False)

    CHS = 132 * (W + 2)          # per-channel element stride in flat xh
    xh = nc.declare_dram_parameter("xh", [CIN * CHS + 2 * (W + 2)], F16,
                                   isOutput=False)
    w1a = nc.declare_dram_parameter("w1a", [128, 6, 128], F16, isOutput=False)
    w1r = nc.declare_dram_parameter("w1r", [128, 2, 128], F16, isOutput=False)
    w1s = nc.declare_dram_parameter("w1s", [128, 2, 128], F16, isOutput=False)
    w2w = nc.declare_dram_parameter("w2w", [128, 48, 128], F16, isOutput=False)
    muw = nc.declare_dram_parameter("muw", [128, 2, K], F16, isOutput=False)
    cst = nc.declare_dram_parameter("cst", [128, 2 * K + 4], F32, isOutput=False)
    outd = nc.declare_dram_parameter("out", [128, W], F32, isOutput=True)

    with tile.TileContext(nc) as tc:
        with tc.tile_pool(name="singles", bufs=1) as singles, \
             tc.tile_pool(name="xpool", bufs=2) as xpool, \
             tc.tile_pool(name="q1pool", bufs=1) as q1pool, \
             tc.tile_pool(name="ttpool", bufs=2) as ttpool, \
             tc.tile_pool(name="ypool", bufs=2) as ypool, \
             tc.tile_pool(name="q2pool", bufs=2) as q2pool, \
             tc.tile_pool(name="smx", bufs=2) as smx, \
             tc.tile_pool(name="obuf", bufs=2) as obuf, \
             tc.tile_pool(name="ps1", bufs=2, space="PSUM") as ps1, \
             tc.tile_pool(name="mps", bufs=1, space="PSUM") as mps, \
             tc.tile_pool(name="psl", bufs=2, space="PSUM") as psl:

            # ---- resident weights (one DMA per family) ----------------
            w1abuf = singles.tile([128, 6, 128], F16, tag="w1abuf")
            nc.sync.dma_start(out=w1abuf, in_=w1a.ap())
            w1a_sb = {(mc, dr): w1abuf[:, mc * 3 + dr, :]
                      for mc in range(2) for dr in range(3)}
            w1rbuf = singles.tile([128, 2, 128], F16, tag="w1rbuf")
            nc.sync.dma_start(out=w1rbuf, in_=w1r.ap())
            w1r_sb = {mc: w1rbuf[:, mc, :] for mc in range(2)}
            w1sbuf = singles.tile([128, 2, 128], F16, tag="w1sbuf")
            nc.sync.dma_start(out=w1sbuf, in_=w1s.ap())
            w1s_sb = {mc: w1sbuf[:, mc, :] for mc in range(2)}
            # band-0 x halo first so conv1 can start before w2 finishes loading
            xh_ap = xh.ap()

            def xsrc(r0, lo, hi, shift):
                # [64ch, rows, cols] slice of the flat xh, shifted by `shift`
                # elements (1 = one column, W+2 = one row)
                return bass.AP(
                    tensor=xh_ap.tensor,
                    offset=(r0 + lo) * (W + 2) + shift,
                    ap=[[CHS, CIN], [W + 2, hi - lo], [1, W + 2]])

            def load_xband(r0, split=False):
                # split=True: row-chunks so band-0 conv1 can start as soon
                # as the first rows land.
                chunks = ([(0, 4), (4, 12), (12, 24), (24, RB + 4)] if split
                          else [(0, RB + 4)])
                xa = xpool.tile([128, RB + 4, W + 2], F16, tag="xa", name="xa")
                xr = xpool.tile([128, RB + 4, W + 2], F16, tag="xr", name="xr")
                for lo, hi in chunks:
                    nc.sync.dma_start(out=xa[0:64, lo:hi, :],
                                      in_=xsrc(r0, lo, hi, 0))
                    nc.sync.dma_start(out=xa[64:128, lo:hi, :],
                                      in_=xsrc(r0, lo, hi, 1))
                    nc.sync.dma_start(out=xr[0:64, lo:hi, :],
                                      in_=xsrc(r0, lo, hi, 0))
                    nc.sync.dma_start(out=xr[64:128, lo:hi, :],
                                      in_=xsrc(r0, lo, hi, W + 2))
                return xa, xr

            xband0 = load_xband(0, split=True)

            # small constants next (before the bulky w2 tiles hog the queues)
            muwbuf = singles.tile([128, 2, K], F16, tag="muwbuf")
            nc.sync.dma_start(out=muwbuf, in_=muw.ap())
            muw_sb = {kc: muwbuf[:, kc, :] for kc in range(2)}
            cstbuf = singles.tile([128, 2 * K + 4], F32, tag="cstbuf")
            nc.sync.dma_start(out=cstbuf, in_=cst.ap())
            bp_sb = cstbuf[:, 0:K]
            lab_sb = cstbuf[:, K:2 * K]
            b1_sb = {mc: cstbuf[:, 2 * K + mc:2 * K + mc + 1] for mc in range(2)}
            b2_sb = {mc: cstbuf[:, 2 * K + 2 + mc:2 * K + 3 + mc] for mc in range(2)}

            # PE warmup: keep TensorE busy through the initial DMA wait so
            # the HAM clock-gate is at 8/8 when real matmuls arrive.
            wscr = singles.tile([128, 512], F16, tag="wscr")
            nc.vector.memset(wscr, 0.0)
            for _ in range(96):
                wps = ps1.tile([128, 2, W], F32, tag="c1ps", name="wps")
                nc.tensor.matmul(wps.rearrange("p a b -> p (a b)"),
                                 wscr[:, 0:128], wscr, start=True, stop=True)

            w2wbuf = singles.tile([128, 48, 128], F16, tag="w2wbuf")
            nc.sync.dma_start(out=w2wbuf, in_=w2w.ap())
            w2w_sb = {(t, dc, kc, mc): w2wbuf[:, ((t * 3 + dc) * 2 + kc) * 2 + mc, :]
                      for t in range(4) for dc in range(3)
                      for kc in range(2) for mc in range(2)}

            # ---- bands ------------------------------------------------
            pend = []     # carried logits/softmax thunks (cross-band)
            for band in range(NBAND):
                r0 = RB * band
                # x halo in two packings:
                #  xa: p0-63 = xh rows r0..r0+35, p64-127 = same shifted +1 col
                #  xr: p0-63 = xh rows,           p64-127 = same shifted +1 row
                xa, xr = xband0 if band == 0 else load_xband(r0)

                # q1 band buffer: slot j = q1 row (r0 - 1 + j), cols 1..256
                # real, cols 0/257 replicate pads.
                q1b = {}
                for mc in range(2):
                    q1b[mc] = q1pool.tile([128, RB + 2, W + 2], F16, tag=f"q1_{mc}", name=f"q1_{mc}")

                # conv1: q1 slot j needs xh local rows j+dr (pairs), and
                # taps (0,2),(1,2) from xr row j, tap (2,2) from xa row j+2
                # (zero-padded to 128 contraction rows).
                if band == 0:
                    groups1 = [(j, 2) for j in range(1, RB + 1, 2)] + [(RB + 1, 1)]
                else:
                    groups1 = [(j, 2) for j in range(0, RB + 2, 2)]
                lo = 1 if band == 0 else 0
                for gi, (j, nr) in enumerate(groups1):
                    for mc in range(2):
                        ps = ps1.tile([128, nr, W], F32, tag="c1ps", name="c1ps")
                        for dr in range(3):
                            nc.tensor.matmul(
                                ps, w1a_sb[mc, dr],
                                xa[:, j + dr:j + dr + nr, 0:W],
                                start=(dr == 0), stop=False)
                        nc.tensor.matmul(ps, w1r_sb[mc],
                                         xr[:, j:j + nr, 2:W + 2],
                                         start=False, stop=False)
                        nc.tensor.matmul(ps, w1s_sb[mc],
                                         xa[:, j + 2:j + 2 + nr, 2:W + 2],
                                         start=False, stop=True)
                        nc.scalar.activation(
                            out=q1b[mc][:, j:j + nr, 1:W + 1], in_=ps,
                            func=AF.Relu, bias=b1_sb[mc], scale=1.0)
                # replicate pads: cols, then (band 0) top row
                for mc in range(2):
                    nc.vector.tensor_copy(
                        out=q1b[mc][:, lo:RB + 2, 0:1],
                        in_=q1b[mc][:, lo:RB + 2, 1:2])
                    nc.vector.tensor_copy(
                        out=q1b[mc][:, lo:RB + 2, W + 1:W + 2],
                        in_=q1b[mc][:, lo:RB + 2, W:W + 1])
                    if band == 0:
                        nc.vector.tensor_copy(
                            out=q1b[mc][:, 0:1, :], in_=q1b[mc][:, 1:2, :])

                # previous band's carried last-pair logits/softmax flush
                # here: PE just streamed this band's conv1, so the q2 it
                # needs has long been ready -> no stall
                for f in pend:
                    f()
                pend.clear()

                # Winograd F(2,3) row-transform tiles: t0=d0-d2, t1=d1+d2,
                # t2=d2-d1, t3=d1-d3 of q1 slots 2p..2p+3 (full 258 cols,
                # col pads included -> dc shifts slice them).
                def build_tt(p):
                    # kc0 on DVE, kc1 on Pool (Pool cannot touch PSUM, but
                    # SBUF->SBUF fp16 ALU is allowed there).
                    tt = {}
                    for kc, eng in ((0, nc.vector), (1, nc.gpsimd)):
                        t = ttpool.tile([128, 4, W + 2], F16, tag=f"tt{kc}",
                                        name=f"tt{kc}")
                        d = q1b[kc]
                        j = 2 * p
                        eng.tensor_tensor(
                            t[:, 0, :], d[:, j, :], d[:, j + 2, :], ALU.subtract)
                        eng.tensor_tensor(
                            t[:, 1, :], d[:, j + 1, :], d[:, j + 2, :], ALU.add)
                        eng.tensor_tensor(
                            t[:, 2, :], d[:, j + 2, :], d[:, j + 1, :], ALU.subtract)
                        eng.tensor_tensor(
                            t[:, 3, :], d[:, j + 1, :], d[:, j + 3, :], ALU.subtract)
                        tt[kc] = t
                    return tt

                ob = obuf.tile([128, RB // 2, 4], F32, tag="ob", name="ob")

                def logits_softmax(q2t, g, ob=ob):
                    # logits: [128 px, K] per 128-px slice, q2 stationary
                    # (ob bound per band: calls may run during the next band)
                    pl = psl.tile([128, 4, K], F32, tag="lps", name="lps")
                    for j in range(4):
                        for kc in range(2):
                            q2flat = q2t[kc].rearrange("p a b -> p (a b)")
                            nc.tensor.matmul(
                                pl[:, j, :], q2flat[:, 128 * j:128 * (j + 1)],
                                muw_sb[kc], start=(kc == 0), stop=(kc == 1))
                    # softmax over K (free axis) + label contraction
                    li = smx.tile([128, 4, K], F32, tag="li", name="li")
                    nc.vector.tensor_tensor(
                        li, pl,
                        bp_sb.unsqueeze(1).to_broadcast([128, 4, K]),
                        ALU.add)
                    mx = smx.tile([128, 4], F32, tag="mx", name="mx")
                    nc.vector.reduce_max(mx, li, axis=AX.X)
                    ls = smx.tile([128, 4, K], F32, tag="ls", name="ls")
                    nc.vector.tensor_tensor(
                        ls, li,
                        mx.unsqueeze(2).to_broadcast([128, 4, K]),
                        ALU.subtract)
                    ex = smx.tile([128, 4, K], F32, tag="ex", name="ex")
                    nc.scalar.activation(out=ex, in_=ls, func=AF.Exp)
                    el = smx.tile([128, 4, K], F32, tag="el", name="el")
                    nc.vector.tensor_tensor(
                        el, ex,
                        lab_sb.unsqueeze(1).to_broadcast([128, 4, K]),
                        ALU.mult)
                    ssum = smx.tile([128, 4], F32, tag="ssum", name="ssum")
                    nc.vector.reduce_sum(ssum, ex, axis=AX.X)
                    wsum = smx.tile([128, 4], F32, tag="wsum", name="wsum")
                    nc.vector.reduce_sum(wsum, el, axis=AX.X)
                    rs = smx.tile([128, 4], F32, tag="rs", name="rs")
                    nc.vector.reciprocal(rs, ssum)
                    nc.vector.tensor_tensor(ob[:, g], wsum, rs, ALU.mult)

                tt_cur = build_tt(0)
                for p in range(RB // 2):
                    # conv2 m-point matmuls: per mc, two 12-MM accumulation
                    # groups, one per 2KB PSUM bank (t-pair 0,1 and 2,3).
                    m = {}
                    for mc in range(2):
                        mp = mps.tile([128, 4, W], F32, tag=f"m{mc}",
                                      name=f"m{mc}")
                        for tlo in (0, 2):
                            n = 0
                            for t in (tlo, tlo + 1):
                                for dc in range(3):
                                    for kc in range(2):
                                        nc.tensor.matmul(
                                            mp[:, t, :],
                                            w2w_sb[t, dc, kc, mc],
                                            tt_cur[kc][:, t, dc:dc + W],
                                            start=(n == 0), stop=(n == 11))
                                        n += 1
                        m[mc] = mp
                    if p + 1 < RB // 2:
                        tt_nxt = build_tt(p + 1)
                    # inverse transform + relu. hw: each ALU op may read
                    # at most ONE input from PSUM -> stage m1 to SBUF via
                    # Scalar, then chain on DVE to free the m-banks fast.
                    q2t = {}
                    for mc in range(2):
                        c1 = ypool.tile([128, W], F32, tag="c1", name="c1")
                        nc.scalar.activation(out=c1, in_=m[mc][:, 1, :],
                                             func=AF.Copy, scale=1.0)
                        y0a = ypool.tile([128, W], F32, tag="y0a", name="y0a")
                        nc.vector.tensor_tensor(
                            y0a, c1, m[mc][:, 0, :], ALU.add)
                        y0 = ypool.tile([128, W], F32, tag="y0", name="y0")
                        nc.vector.tensor_tensor(
                            y0, y0a, m[mc][:, 2, :], ALU.add)
                        y1a = ypool.tile([128, W], F32, tag="y1a", name="y1a")
                        nc.vector.tensor_tensor(
                            y1a, c1, m[mc][:, 2, :], ALU.subtract)
                        y1 = ypool.tile([128, W], F32, tag="y1", name="y1")
                        nc.vector.tensor_tensor(
                            y1, y1a, m[mc][:, 3, :], ALU.subtract)
                        q2t[mc] = q2pool.tile([128, 2, W], F16, tag=f"q2_{mc}",
                                              name=f"q2_{mc}")
                        nc.scalar.activation(out=q2t[mc][:, 0, :], in_=y0,
                                             func=AF.Relu, bias=b2_sb[mc],
                                             scale=1.0)
                        nc.scalar.activation(out=q2t[mc][:, 1, :], in_=y1,
                                             func=AF.Relu, bias=b2_sb[mc],
                                             scale=1.0)
                    # logits for the PREVIOUS pair: issued after this pair's
                    # conv matmuls so the PE never waits on scalar/vector.
                    if pend:
                        pend.pop(0)()
                    def make_entry(ls=logits_softmax, q=q2t, g=p,
                                   bob=ob, br0=r0):
                        def run():
                            ls(q, g)
                            if g in (RB // 4 - 1, RB // 2 - 1):
                                # half of the band's rows done -> ship them
                                half = 0 if g == RB // 4 - 1 else 1
                                h = RB // 2
                                nc.sync.dma_start(
                                    out=outd.ap()[br0 + half * h:
                                                  br0 + half * h + h, :]
                                    .rearrange("(g r) (jj p) -> p g r jj",
                                               r=2, p=128),
                                    in_=bob[:, half * RB // 4:
                                            half * RB // 4 + RB // 4]
                                    .rearrange("p g (r jj) -> p g r jj", r=2))
                        return run
                    pend.append(make_entry())
                    if p + 1 < RB // 2:
                        tt_cur = tt_nxt
                if band == NBAND - 1:
                    for f in pend:
                        f()
                    pend.clear()

    nc.compile()
    return nc


def prep_inputs(x, w1, b1, w2, b2, w3, b3, mu, label):
    """Full inputs -> per-core in_maps."""
    w3m = w3[:, :, 0, 0]
    muW = 2.0 * (mu @ w3m)                                   # [K, Q]
    bpv = (2.0 * (mu @ b3) - (mu * mu).sum(1)).astype(np.float32)

    def pack_w(w1f, w2f):
        w1ap = np.empty((2, 3, 128, 128), np.float32)
        w1rp = np.empty((2, 128, 128), np.float32)
        w1sp = np.zeros((2, 128, 128), np.float32)
        for mc in range(2):
            ms = slice(128 * mc, 128 * (mc + 1))
            for dr in range(3):
                w1ap[mc, dr, 0:64] = w1f[ms, :, dr, 0].T
                w1ap[mc, dr, 64:128] = w1f[ms, :, dr, 1].T
            w1rp[mc, 0:64] = w1f[ms, :, 0, 2].T
            w1rp[mc, 64:128] = w1f[ms, :, 1, 2].T
            w1sp[mc, 0:64] = w1f[ms, :, 2, 2].T
        # conv2 weights: Winograd F(2,3) G-transform over dr, per (t, dc,
        # kc, mc): g0=W0, g1=(W0+W1+W2)/2, g2=(W0-W1+W2)/2, g3=W2
        w2p = np.empty((48, 128, 128), np.float32)
        for t in range(4):
            for dc in range(3):
                for kc in range(2):
                    for mc in range(2):
                        blk = [w2f[128 * mc:128 * (mc + 1),
                                   128 * kc:128 * (kc + 1), dr, dc].T
                               for dr in range(3)]
                        if t == 0:
                            g = blk[0]
                        elif t == 1:
                            g = 0.5 * (blk[0] + blk[1] + blk[2])
                        elif t == 2:
                            g = 0.5 * (blk[0] - blk[1] + blk[2])
                        else:
                            g = blk[2]
                        w2p[((t * 3 + dc) * 2 + kc) * 2 + mc] = g
        return (np.ascontiguousarray(w1ap.reshape(6, 128, 128).transpose(1, 0, 2)).astype(np.float16),
                np.ascontiguousarray(w1rp.transpose(1, 0, 2)).astype(np.float16),
                np.ascontiguousarray(w1sp.transpose(1, 0, 2)).astype(np.float16),
                np.ascontiguousarray(w2p.transpose(1, 0, 2)).astype(np.float16))

    packs = {}
    packs[0] = pack_w(w1, w2)
    packs[1] = pack_w(w1[:, :, ::-1, :], w2[:, :, ::-1, :])

    muwp = np.empty((128, 2, K), np.float32)
    for kc in range(2):
        muwp[:, kc, :] = muW[:, 128 * kc:128 * (kc + 1)].T
    muwp = muwp.astype(np.float16)
    cstv = np.empty((128, 2 * K + 4), np.float32)
    cstv[:, 0:K] = bpv[None, :]
    cstv[:, K:2 * K] = label[None, :].astype(np.float32)
    for mc in range(2):
        cstv[:, 2 * K + mc] = b1[128 * mc:128 * (mc + 1)]
        cstv[:, 2 * K + 2 + mc] = b2[128 * mc:128 * (mc + 1)]

    rows = np.clip(np.arange(132) - 2, 0, H - 1)
    cols = np.clip(np.arange(W + 2) - 1, 0, W - 1)
    in_maps = []
    for core in range(NCORES):
        img, half = core // 2, core % 2
        xl = x[img] if half == 0 else x[img, :, ::-1, :]
        xhv = np.ascontiguousarray(xl[:, rows][:, :, cols]).astype(np.float16)
        xhf = np.concatenate([xhv.reshape(-1),
                              np.zeros(2 * (W + 2), np.float16)])
        w1ap, w1rp, w1sp, w2p = packs[half]
        in_maps.append({
            'xh': xhf, 'w1a': w1ap, 'w1r': w1rp, 'w1s': w1sp, 'w2w': w2p,
            'muw': muwp, 'cst': cstv,
        })
    return in_maps


def gather(results, dtype=np.float32):
    out = np.empty((B, 1, H, W), dtype)
    for core in range(NCORES):
        img, half = core // 2, core % 2
        o = results[core]['out']
        if half == 0:
            out[img, 0, 0:128] = o
        else:
            out[img, 0, 128:256] = o[::-1]
    return out


def get_nc():
    if 'nc' not in _cached:
        _cached['nc'] = build_nc()
    return _cached['nc']


def kernel(x, w1, b1, w2, b2, w3, b3, mu, label, **run_kwargs):
    nc = get_nc()
    in_maps = prep_inputs(
        np.asarray(x, np.float32), np.asarray(w1, np.float32),
        np.asarray(b1, np.float32), np.asarray(w2, np.float32),
        np.asarray(b2, np.float32), np.asarray(w3, np.float32),
        np.asarray(b3, np.float32), np.asarray(mu, np.float32),
        np.asarray(label, np.float32))
    res = run_bass_kernel_spmd(nc, in_maps, core_ids=list(range(NCORES)),
                               **run_kwargs)
    out = gather(res.results)
    if run_kwargs:
        _cached['last_result'] = res
    return out


# revision 35
# speedup vs baseline: 1.0072x; 1.0072x over previous
"""AttentionClustering (vq_codebook) Trainium2 kernel, 8-core data parallel.

Shard: 8 cores = 4 images x 2 half-images (128 output rows each). Odd cores
get a vertically flipped shard + row-flipped conv weights so every core's
program is identical (true image edge at local top, interior halo at bottom).

Math: q1 = relu(conv3x3(x, w1) + b1); q2 = relu(conv3x3(q1, w2) + b2)  (both
with replicate padding); then the 1x1 conv + cluster-distance softmax folds to
  logit[px, k] = sum_ci q2[ci, px] * muW[k, ci] + bp[k]
  muW = 2 * mu @ W3,  bp = 2 * mu @ b3 - |mu|^2      (|q|^2 cancels in softmax)
  out[px] = sum_k softmax_k(logit) * label[k]

conv2 runs as Winograd F(2,3) over the row (dr) axis: output rows (2p, 2p+1)
come from 4 t-combinations of q1 rows 2p-1..2p+2 (built on DVE), contracted
against G-transformed weights (4t x 3dc x 2kc matmuls of 256 free instead of
direct 2x9 of 512 free -> 2/3 the PE cycles), then the A^T inverse transform
y0 = m0+m1+m2 (DVE) / y1 = m1-m2-m3 (Pool) and the relu+bias on Scalar.
PSUM m-tiles pack t-pairs per 2KB bank, so (t0,t1) and (t2,t3) are emitted as
single 12-matmul accumulation groups (bank zero-regions materialize lazily).
conv1 stays direct fp16; its (2,2) tap is zero-padded to 128 contraction rows
to avoid the 64-row PE tile reconfig penalty. Logits for pair p issue after
conv matmuls of pair p+1 so the PE never waits on the softmax chain.
"""
import sys
if '/opt/trn_rl_repo' not in sys.path:
    sys.path.insert(0, '/opt/trn_rl_repo')

import numpy as np
import concourse.bass as bass
import concourse.mybir as mybir
from concourse import bacc, tile
from concourse.bass_utils import run_bass_kernel_spmd

F32 = mybir.dt.float32
F16 = mybir.dt.float16
AF = mybir.ActivationFunctionType
ALU = mybir.AluOpType
AX = mybir.AxisListType

B, CIN, H, W = 4, 64, 256, 256
Q, K = 256, 16
RB = 32           # output rows per band
NBAND = 4         # bands per core (128 rows)
NCORES = 8

_cached = {}


def build_nc():
    nc = bacc.Bacc("TRN2", target_bir_lowering=False, debug=False)

    CHS = 132 * (W + 2)          # per-channel element stride in flat xh
    xh = nc.declare_dram_parameter("xh", [CIN * CHS + 2 * (W + 2)], F16,
                                   isOutput=False)
    w1a = nc.declare_dram_parameter("w1a", [128, 6, 128], F16, isOutput=False)
    w1r = nc.declare_dram_parameter("w1r", [128, 2, 128], F16, isOutput=False)
    w1s = nc.declare_dram_parameter("w1s", [128, 2, 128], F16, isOutput=False)
    w2w = nc.declare_dram_parameter("w2w", [128, 48, 128], F16, isOutput=False)
    muw = nc.declare_dram_parameter("muw", [128, 2, K], F16, isOutput=False)
    cst = nc.declare_dram_parameter("cst", [128, 2 * K + 4], F32, isOutput=False)
    outd = nc.declare_dram_parameter("out", [128, W], F32, isOutput=True)

    with tile.TileContext(nc) as tc:
        with tc.tile_pool(name="singles", bufs=1) as singles, \
             tc.tile_pool(name="xpool", bufs=2) as xpool, \
             tc.tile_pool(name="q1pool", bufs=1) as q1pool, \
             tc.tile_pool(name="ttpool", bufs=2) as ttpool, \
             tc.tile_pool(name="ypool", bufs=2) as ypool, \
             tc.tile_pool(name="q2pool", bufs=2) as q2pool, \
             tc.tile_pool(name="smx", bufs=2) as smx, \
             tc.tile_pool(name="obuf", bufs=2) as obuf, \
             tc.tile_pool(name="ps1", bufs=2, space="PSUM") as ps1, \
             tc.tile_pool(name="mps", bufs=1, space="PSUM") as mps, \
             tc.tile_pool(name="psl", bufs=2, space="PSUM") as psl:

            # ---- resident weights (one DMA per family) ----------------
            w1abuf = singles.tile([128, 6, 128], F16, tag="w1abuf")
            nc.sync.dma_start(out=w1abuf, in_=w1a.ap())
            w1a_sb = {(mc, dr): w1abuf[:, mc * 3 + dr, :]
                      for mc in range(2) for dr in range(3)}
            w1rbuf = singles.tile([128, 2, 128], F16, tag="w1rbuf")
            nc.sync.dma_start(out=w1rbuf, in_=w1r.ap())
            w1r_sb = {mc: w1rbuf[:, mc, :] for mc in range(2)}
            w1sbuf = singles.tile([128, 2, 128], F16, tag="w1sbuf")
            nc.sync.dma_start(out=w1sbuf, in_=w1s.ap())
            w1s_sb = {mc: w1sbuf[:, mc, :] for mc in range(2)}
            # band-0 x halo first so conv1 can start before w2 finishes loading
            xh_ap = xh.ap()

            def xsrc(r0, lo, hi, shift):
                # [64ch, rows, cols] slice of the flat xh, shifted by `shift`
                # elements (1 = one column, W+2 = one row)
                return bass.AP(
                    tensor=xh_ap.tensor,
                    offset=(r0 + lo) * (W + 2) + shift,
                    ap=[[CHS, CIN], [W + 2, hi - lo], [1, W + 2]])

            def load_xband(r0, split=False):
                # split=True: row-chunks so band-0 conv1 can start as soon
                # as the first rows land.
                chunks = ([(0, 4), (4, 12), (12, 24), (24, RB + 4)] if split
                          else [(0, RB + 4)])
                xa = xpool.tile([128, RB + 4, W + 2], F16, tag="xa", name="xa")
                xr = xpool.tile([128, RB + 4, W + 2], F16, tag="xr", name="xr")
                for lo, hi in chunks:
                    nc.sync.dma_start(out=xa[0:64, lo:hi, :],
                                      in_=xsrc(r0, lo, hi, 0))
                    nc.sync.dma_start(out=xa[64:128, lo:hi, :],
                                      in_=xsrc(r0, lo, hi, 1))
                    nc.sync.dma_start(out=xr[0:64, lo:hi, :],
                                      in_=xsrc(r0, lo, hi, 0))
                    nc.sync.dma_start(out=xr[64:128, lo:hi, :],
                                      in_=xsrc(r0, lo, hi, W + 2))
                return xa, xr

            xband0 = load_xband(0, split=True)

            # small constants next (before the bulky w2 tiles hog the queues)
            muwbuf = singles.tile([128, 2, K], F16, tag="muwbuf")
            nc.sync.dma_start(out=muwbuf, in_=muw.ap())
            muw_sb = {kc: muwbuf[:, kc, :] for kc in range(2)}
            cstbuf = singles.tile([128, 2 * K + 4], F32, tag="cstbuf")
            nc.sync.dma_start(out=cstbuf, in_=cst.ap())
            bp_sb = cstbuf[:, 0:K]
            lab_sb = cstbuf[:, K:2 * K]
            b1_sb = {mc: cstbuf[:, 2 * K + mc:2 * K + mc + 1] for mc in range(2)}
            b2_sb = {mc: cstbuf[:, 2 * K + 2 + mc:2 * K + 3 + mc] for mc in range(2)}

            # PE warmup: keep TensorE busy through the initial DMA wait so
            # the HAM clock-gate is at 8/8 when real matmuls arrive.
            wscr = singles.tile([128, 512], F16, tag="wscr")
            nc.vector.memset(wscr, 0.0)
            for _ in range(96):
                wps = ps1.tile([128, 2, W], F32, tag="c1ps", name="wps")
                nc.tensor.matmul(wps.rearrange("p a b -> p (a b)"),
                                 wscr[:, 0:128], wscr, start=True, stop=True)

            w2wbuf = singles.tile([128, 48, 128], F16, tag="w2wbuf")
            nc.sync.dma_start(out=w2wbuf, in_=w2w.ap())
            w2w_sb = {(t, dc, kc, mc): w2wbuf[:, ((t * 3 + dc) * 2 + kc) * 2 + mc, :]
                      for t in range(4) for dc in range(3)
                      for kc in range(2) for mc in range(2)}

            # ---- bands ------------------------------------------------
            pend = []     # carried logits/softmax thunks (cross-band)
            for band in range(NBAND):
                r0 = RB * band
                # x halo in two packings:
                #  xa: p0-63 = xh rows r0..r0+35, p64-127 = same shifted +1 col
                #  xr: p0-63 = xh rows,           p64-127 = same shifted +1 row
                xa, xr = xband0 if band == 0 else load_xband(r0)

                # q1 band buffer: slot j = q1 row (r0 - 1 + j), cols 1..256
                # real, cols 0/257 replicate pads.
                q1b = {}
                for mc in range(2):
                    q1b[mc] = q1pool.tile([128, RB + 2, W + 2], F16, tag=f"q1_{mc}", name=f"q1_{mc}")

                # conv1: q1 slot j needs xh local rows j+dr (pairs), and
                # taps (0,2),(1,2) from xr row j, tap (2,2) from xa row j+2
                # (zero-padded to 128 contraction rows).
                if band == 0:
                    groups1 = [(j, 2) for j in range(1, RB + 1, 2)] + [(RB + 1, 1)]
                else:
                    groups1 = [(j, 2) for j in range(0, RB + 2, 2)]
                lo = 1 if band == 0 else 0
                for gi, (j, nr) in enumerate(groups1):
                    for mc in range(2):
                        ps = ps1.tile([128, nr, W], F32, tag="c1ps", name="c1ps")
                        for dr in range(3):
                            nc.tensor.matmul(
                                ps, w1a_sb[mc, dr],
                                xa[:, j + dr:j + dr + nr, 0:W],
                                start=(dr == 0), stop=False)
                        nc.tensor.matmul(ps, w1r_sb[mc],
                                         xr[:, j:j + nr, 2:W + 2],
                                         start=False, stop=False)
                        nc.tensor.matmul(ps, w1s_sb[mc],
                                         xa[:, j + 2:j + 2 + nr, 2:W + 2],
                                         start=False, stop=True)
                        nc.scalar.activation(
                            out=q1b[mc][:, j:j + nr, 1:W + 1], in_=ps,
                            func=AF.Relu, bias=b1_sb[mc], scale=1.0)
                # replicate pads: cols, then (band 0) top row
                for mc in range(2):
                    nc.vector.tensor_copy(
                        out=q1b[mc][:, lo:RB + 2, 0:1],
                        in_=q1b[mc][:, lo:RB + 2, 1:2])
                    nc.vector.tensor_copy(
                        out=q1b[mc][:, lo:RB + 2, W + 1:W + 2],
                        in_=q1b[mc][:, lo:RB + 2, W:W + 1])
                    if band == 0:
                        nc.vector.tensor_copy(
                            out=q1b[mc][:, 0:1, :], in_=q1b[mc][:, 1:2, :])

                # Winograd F(2,3) row-transform tiles: t0=d0-d2, t1=d1+d2,
                # t2=d2-d1, t3=d1-d3 of q1 slots 2p..2p+3 (full 258 cols,
                # col pads included -> dc shifts slice them).
                def build_tt(p):
                    # kc0 on DVE, kc1 on Pool (Pool cannot touch PSUM, but
                    # SBUF->SBUF fp16 ALU is allowed there).
                    tt = {}
                    for kc, eng in ((0, nc.vector), (1, nc.gpsimd)):
                        t = ttpool.tile([128, 4, W + 2], F16, tag=f"tt{kc}",
                                        name=f"tt{kc}")
                        d = q1b[kc]
                        j = 2 * p
                        eng.tensor_tensor(
                            t[:, 0, :], d[:, j, :], d[:, j + 2, :], ALU.subtract)
                        eng.tensor_tensor(
                            t[:, 1, :], d[:, j + 1, :], d[:, j + 2, :], ALU.add)
                        eng.tensor_tensor(
                            t[:, 2, :], d[:, j + 2, :], d[:, j + 1, :], ALU.subtract)
                        eng.tensor_tensor(
                            t[:, 3, :], d[:, j + 1, :], d[:, j + 3, :], ALU.subtract)
                        tt[kc] = t
                    return tt

                ob = obuf.tile([128, RB // 2, 4], F32, tag="ob", name="ob")

                def logits_softmax(q2t, g, ob=ob):
                    # logits: [128 px, K] per 128-px slice, q2 stationary
                    # (ob bound per band: calls may run during the next band)
                    pl = psl.tile([128, 4, K], F32, tag="lps", name="lps")
                    for j in range(4):
                        for kc in range(2):
                            q2flat = q2t[kc].rearrange("p a b -> p (a b)")
                            nc.tensor.matmul(
                                pl[:, j, :], q2flat[:, 128 * j:128 * (j + 1)],
                                muw_sb[kc], start=(kc == 0), stop=(kc == 1))
                    # softmax over K (free axis) + label contraction
                    li = smx.tile([128, 4, K], F32, tag="li", name="li")
                    nc.vector.tensor_tensor(
                        li, pl,
                        bp_sb.unsqueeze(1).to_broadcast([128, 4, K]),
                        ALU.add)
                    mx = smx.tile([128, 4], F32, tag="mx", name="mx")
                    nc.vector.reduce_max(mx, li, axis=AX.X)
                    ls = smx.tile([128, 4, K], F32, tag="ls", name="ls")
                    nc.vector.tensor_tensor(
                        ls, li,
                        mx.unsqueeze(2).to_broadcast([128, 4, K]),
                        ALU.subtract)
                    ex = smx.tile([128, 4, K], F32, tag="ex", name="ex")
                    nc.scalar.activation(out=ex, in_=ls, func=AF.Exp)
                    el = smx.tile([128, 4, K], F32, tag="el", name="el")
                    nc.vector.tensor_tensor(
                        el, ex,
                        lab_sb.unsqueeze(1).to_broadcast([128, 4, K]),
                        ALU.mult)
                    ssum = smx.tile([128, 4], F32, tag="ssum", name="ssum")
                    nc.vector.reduce_sum(ssum, ex, axis=AX.X)
                    wsum = smx.tile([128, 4], F32, tag="wsum", name="wsum")
                    nc.vector.reduce_sum(wsum, el, axis=AX.X)
                    rs = smx.tile([128, 4], F32, tag="rs", name="rs")
                    nc.vector.reciprocal(rs, ssum)
                    nc.vector.tensor_tensor(ob[:, g], wsum, rs, ALU.mult)

                tt_cur = build_tt(0)
                for p in range(RB // 2):
                    # conv2 m-point matmuls: per mc, two 12-MM accumulation
                    # groups, one per 2KB PSUM bank (t-pair 0,1 and 2,3).
                    m = {}
                    for mc in range(2):
                        mp = mps.tile([128, 4, W], F32, tag=f"m{mc}",
                                      name=f"m{mc}")
                        for tlo in (0, 2):
                            n = 0
                            for t in (tlo, tlo + 1):
                                for dc in range(3):
                                    for kc in range(2):
                                        nc.tensor.matmul(
                                            mp[:, t, :],
                                            w2w_sb[t, dc, kc, mc],
                                            tt_cur[kc][:, t, dc:dc + W],
                                            start=(n == 0), stop=(n == 11))
                                        n += 1
                        m[mc] = mp
                    if p + 1 < RB // 2:
                        tt_nxt = build_tt(p + 1)
                    # inverse transform + relu. hw: each ALU op may read
                    # at most ONE input from PSUM -> stage m1 to SBUF via
                    # Scalar, then chain on DVE to free the m-banks fast.
                    q2t = {}
                    for mc in range(2):
                        c1 = ypool.tile([128, W], F32, tag="c1", name="c1")
                        nc.scalar.activation(out=c1, in_=m[mc][:, 1, :],
                                             func=AF.Copy, scale=1.0)
                        y0a = ypool.tile([128, W], F32, tag="y0a", name="y0a")
                        nc.vector.tensor_tensor(
                            y0a, c1, m[mc][:, 0, :], ALU.add)
                        y0 = ypool.tile([128, W], F32, tag="y0", name="y0")
                        nc.vector.tensor_tensor(
                            y0, y0a, m[mc][:, 2, :], ALU.add)
                        y1a = ypool.tile([128, W], F32, tag="y1a", name="y1a")
                        nc.vector.tensor_tensor(
                            y1a, c1, m[mc][:, 2, :], ALU.subtract)
                        y1 = ypool.tile([128, W], F32, tag="y1", name="y1")
                        nc.vector.tensor_tensor(
                            y1, y1a, m[mc][:, 3, :], ALU.subtract)
                        q2t[mc] = q2pool.tile([128, 2, W], F16, tag=f"q2_{mc}",
                                              name=f"q2_{mc}")
                        nc.scalar.activation(out=q2t[mc][:, 0, :], in_=y0,
                                             func=AF.Relu, bias=b2_sb[mc],
                                             scale=1.0)
                        nc.scalar.activation(out=q2t[mc][:, 1, :], in_=y1,
                                             func=AF.Relu, bias=b2_sb[mc],
                                             scale=1.0)
                    # logits for the PREVIOUS pair: issued after this pair's
                    # conv matmuls so the PE never waits on scalar/vector.
                    if pend:
                        pend.pop(0)()
                    def make_entry(ls=logits_softmax, q=q2t, g=p,
                                   bob=ob, br0=r0):
                        def run():
                            ls(q, g)
                            if g in (RB // 4 - 1, RB // 2 - 1):
                                # half of the band's rows done -> ship them
                                half = 0 if g == RB // 4 - 1 else 1
                                h = RB // 2
                                nc.sync.dma_start(
                                    out=outd.ap()[br0 + half * h:
                                                  br0 + half * h + h, :]
                                    .rearrange("(g r) (jj p) -> p g r jj",
                                               r=2, p=128),
                                    in_=bob[:, half * RB // 4:
                                            half * RB // 4 + RB // 4]
                                    .rearrange("p g (r jj) -> p g r jj", r=2))
                        return run
                    pend.append(make_entry())
                    if p + 1 < RB // 2:
                        tt_cur = tt_nxt
                # flush the band's last pair before leaving the band
                for f in pend:
                    f()
                pend.clear()

    nc.compile()
    return nc


def prep_inputs(x, w1, b1, w2, b2, w3, b3, mu, label):
    """Full inputs -> per-core in_maps."""
    w3m = w3[:, :, 0, 0]
    muW = 2.0 * (mu @ w3m)                                   # [K, Q]
    bpv = (2.0 * (mu @ b3) - (mu * mu).sum(1)).astype(np.float32)

    def pack_w(w1f, w2f):
        w1ap = np.empty((2, 3, 128, 128), np.float32)
        w1rp = np.empty((2, 128, 128), np.float32)
        w1sp = np.zeros((2, 128, 128), np.float32)
        for mc in range(2):
            ms = slice(128 * mc, 128 * (mc + 1))
            for dr in range(3):
                w1ap[mc, dr, 0:64] = w1f[ms, :, dr, 0].T
                w1ap[mc, dr, 64:128] = w1f[ms, :, dr, 1].T
            w1rp[mc, 0:64] = w1f[ms, :, 0, 2].T
            w1rp[mc, 64:128] = w1f[ms, :, 1, 2].T
            w1sp[mc, 0:64] = w1f[ms, :, 2, 2].T
        # conv2 weights: Winograd F(2,3) G-transform over dr, per (t, dc,
        # kc, mc): g0=W0, g1=(W0+W1+W2)/2, g2=(W0-W1+W2)/2, g3=W2
        w2p = np.empty((48, 128, 128), np.float32)
        for t in range(4):
            for dc in range(3):
                for kc in range(2):
                    for mc in range(2):
                        blk = [w2f[128 * mc:128 * (mc + 1),
                                   128 * kc:128 * (kc + 1), dr, dc].T
                               for dr in range(3)]
                        if t == 0:
                            g = blk[0]
                        elif t == 1:
                            g = 0.5 * (blk[0] + blk[1] + blk[2])
                        elif t == 2:
                            g = 0.5 * (blk[0] - blk[1] + blk[2])
                        else:
                            g = blk[2]
                        w2p[((t * 3 + dc) * 2 + kc) * 2 + mc] = g
        return (np.ascontiguousarray(w1ap.reshape(6, 128, 128).transpose(1, 0, 2)).astype(np.float16),
                np.ascontiguousarray(w1rp.transpose(1, 0, 2)).astype(np.float16),
                np.ascontiguousarray(w1sp.transpose(1, 0, 2)).astype(np.float16),
                np.ascontiguousarray(w2p.transpose(1, 0, 2)).astype(np.float16))

    packs = {}
    packs[0] = pack_w(w1, w2)
    packs[1] = pack_w(w1[:, :, ::-1, :], w2[:, :, ::-1, :])

    muwp = np.empty((128, 2, K), np.float32)
    for kc in range(2):
        muwp[:, kc, :] = muW[:, 128 * kc:128 * (kc + 1)].T
    muwp = muwp.astype(np.float16)
    cstv = np.empty((128, 2 * K + 4), np.float32)
    cstv[:, 0:K] = bpv[None, :]
    cstv[:, K:2 * K] = label[None, :].astype(np.float32)
    for mc in range(2):
        cstv[:, 2 * K + mc] = b1[128 * mc:128 * (mc + 1)]
        cstv[:, 2 * K + 2 + mc] = b2[128 * mc:128 * (mc + 1)]

    rows = np.clip(np.arange(132) - 2, 0, H - 1)
    cols = np.clip(np.arange(W + 2) - 1, 0, W - 1)
    in_maps = []
    for core in range(NCORES):
        img, half = core // 2, core % 2
        xl = x[img] if half == 0 else x[img, :, ::-1, :]
        xhv = np.ascontiguousarray(xl[:, rows][:, :, cols]).astype(np.float16)
        xhf = np.concatenate([xhv.reshape(-1),
                              np.zeros(2 * (W + 2), np.float16)])
        w1ap, w1rp, w1sp, w2p = packs[half]
        in_maps.append({
            'xh': xhf, 'w1a': w1ap, 'w1r': w1rp, 'w1s': w1sp, 'w2w': w2p,
            'muw': muwp, 'cst': cstv,
        })
    return in_maps


def gather(results, dtype=np.float32):
    out = np.empty((B, 1, H, W), dtype)
    for core in range(NCORES):
        img, half = core // 2, core % 2
        o = results[core]['out']
        if half == 0:
            out[img, 0, 0:128] = o
        else:
            out[img, 0, 128:256] = o[::-1]
    return out


def get_nc():
    if 'nc' not in _cached:
        _cached['nc'] = build_nc()
    return _cached['nc']


def kernel(x, w1, b1, w2, b2, w3, b3, mu, label, **run_kwargs):
    nc = get_nc()
    in_maps = prep_inputs(
        np.asarray(x, np.float32), np.asarray(w1, np.float32),
        np.asarray(b1, np.float32), np.asarray(w2, np.float32),
        np.asarray(b2, np.float32), np.asarray(w3, np.float32),
        np.asarray(b3, np.float32), np.asarray(mu, np.float32),
        np.asarray(label, np.float32))
    res = run_bass_kernel_spmd(nc, in_maps, core_ids=list(range(NCORES)),
                               **run_kwargs)
    out = gather(res.results)
    if run_kwargs:
        _cached['last_result'] = res
    return out
